# revision 35
# baseline (speedup 1.0000x reference)
"""GQA attention (B=2, S=2048, DM=1024, H=16, KH=4, RoPE, causal) on 8 TRN2 cores.

Sharding: DP=2 over batch x TP=4 over heads. Core c handles batch c//4 and
q-heads [4r, 4r+4), kv-head r, where r = c % 4. Each core computes a partial
out^T = wo_shard @ attn_shard of shape [DM, S] in bf16; the host sums the 4
partials per batch in f32 and transposes (gather/unshard).

Per-core kernel (single NEFF, SPMD):
  - Q/K feature-major via transposed weight layouts prepared on host; V
    PE-transposed to token-major with a ones column appended (rowsum trick).
  - RoPE: adjacent-partition swap via stream_shuffle + elementwise ops.
  - Causal mask applied PRE-exp as a PE matmul accumulate of -300 onto the
    128x128 diagonal triangles (identity stationary x const triangle moving),
    so exp() output is ~0 there and no post-exp masking is needed.
  - Scores matmuls run two heads concurrently in the two 64-row PE groups
    (K duplicated to partitions 64..127); AV lags one key-block so the PE
    never head-of-line blocks on the current exp. The PE clock (HAM) is
    pre-warmed with dummy matmuls while the first DMAs land.
  - Softmax division: denominator row reshaped across partitions via two
    small DMAs, DVE reciprocal, gpsimd partition_broadcast + multiplies.
  - Projection and out-projection work is chopped into small "filler" items
    that are emitted between attention key-blocks so the PE never runs a
    long non-attention burst while the ACT engine starves.
"""

from collections import deque

import numpy as np
import ml_dtypes

import concourse.bass as bass
import concourse.mybir as mybir
import concourse.tile as tile
from concourse import bacc
from concourse.bass_utils import run_bass_kernel_spmd
from concourse.masks import make_identity

F32 = mybir.dt.float32
BF16 = mybir.dt.bfloat16

B, S, DM, H, KH, HD = 2, 2048, 1024, 16, 4, 64
N_CORES = 8
TPG = 4                 # tensor-parallel group size
QH = H // TPG           # q-heads per core
KFEAT = QH * HD         # 256 q-features per core
SC = 512                # token chunk
NCH = S // SC           # 4
KB = 128                # key block
NKB = S // KB           # 16
SCALE = 1.0 / np.sqrt(HD)
MBIG = -300.0           # pre-exp additive mask value
XOR1 = [i ^ 1 for i in range(32)]

LAST_RESULTS = None     # BassKernelResults of the most recent run (for test.py)
_NC_CACHE = None


def build_nc():
    nc = bacc.Bacc("TRN2", target_bir_lowering=False, debug=False,
                   num_devices=N_CORES)

    # all inputs pre-packed on host so each partition line is contiguous
    xP = nc.declare_dram_parameter("xP", [128, NCH, 8, SC], BF16, isOutput=False)
    wqP = nc.declare_dram_parameter("wqP", [128, 8, KFEAT], BF16, isOutput=False)
    wkvP = nc.declare_dram_parameter("wkvP", [128, 8, 128], BF16, isOutput=False)
    woP = nc.declare_dram_parameter("woP", [128, 2, DM], BF16, isOutput=False)
    ropeCos = nc.declare_dram_parameter("ropeCos", [128, S], F32, isOutput=False)
    ropeSin = nc.declare_dram_parameter("ropeSin", [128, S], F32, isOutput=False)
    maskM = nc.declare_dram_parameter("maskM", [128, 2, 128], BF16, isOutput=False)
    out = nc.declare_dram_parameter("out", [128, NCH, 8, SC], BF16, isOutput=True)

    EXP = mybir.ActivationFunctionType.Exp
    MUL = bass.mybir.AluOpType.mult
    ADD = bass.mybir.AluOpType.add

    with tile.TileContext(nc) as tc:
        with (
            tc.tile_pool(name="consts", bufs=1) as consts,
            tc.tile_pool(name="kch", bufs=NCH) as kch_pool,
            tc.tile_pool(name="qch", bufs=NCH) as qch_pool,
            tc.tile_pool(name="ach", bufs=2 * NCH) as ach_pool,
            tc.tile_pool(name="v1p", bufs=NKB) as v1_pool,
            tc.tile_pool(name="xch", bufs=NCH) as xch_pool,
            tc.tile_pool(name="tmp", bufs=3) as tmp_pool,
            tc.tile_pool(name="pp", bufs=8) as p_pool,
            tc.tile_pool(name="ocp", bufs=3) as oc_pool,
            tc.tile_pool(name="rp", bufs=2) as r_pool,
            tc.tile_pool(name="bcp", bufs=4) as bc_pool,
            tc.tile_pool(name="op", bufs=2) as o_pool,
            tc.tile_pool(name="acc", bufs=2, space="PSUM") as acc_pool,
            tc.tile_pool(name="oac", bufs=1, space="PSUM") as oacc_pool,
            tc.tile_pool(name="sme", bufs=2, space="PSUM") as s_pool,
        ):
            # ---- constants (ordered for startup latency) ----
            wq_sb = consts.tile([128, 8, KFEAT], BF16, tag="wq")
            wkv_sb = consts.tile([128, 8, 128], BF16, tag="wkv")
            wo_sb = consts.tile([128, 2, DM], BF16, tag="wo")
            cos_sb = consts.tile([128, S], F32, tag="cos")
            sin_sb = consts.tile([128, S], F32, tag="sin")
            mask_sb = consts.tile([128, 2, 128], BF16, tag="mask")
            ident = consts.tile([128, 128], BF16, tag="ident")
            ones_b = consts.tile([128, 64], BF16, tag="onesb")

            def load_x(c0, eng=None, split=False):
                xt = xch_pool.tile([128, 8, SC], BF16, tag="x",
                                   name=f"x_c{c0}")
                e = eng or nc.sync
                if split:
                    # halves on different HWDGE queues -> parallel transfer
                    e.dma_start(xt[:, 0:4, :], xP[:, c0, 0:4, :])
                    nc.sync.dma_start(xt[:, 4:8, :], xP[:, c0, 4:8, :])
                else:
                    e.dma_start(xt[:], xP[:, c0, :, :])
                return xt

            with tc.high_priority():
                nc.sync.dma_start(wq_sb[:], wqP[:])
                nc.sync.dma_start(wkv_sb[:], wkvP[:])
                X_ch = [load_x(0, eng=nc.scalar, split=True)]
                QS = S // 4
                nc.sync.dma_start(mask_sb[:], maskM[:])
                nc.sync.dma_start(cos_sb[:, 0:QS], ropeCos[:, 0:QS])
                nc.sync.dma_start(sin_sb[:, 0:QS], ropeSin[:, 0:QS])
                make_identity(nc, ident[:])
                nc.vector.memset(ones_b[:], 1.0)
                # warm up the PE clock (HAM) while the first DMAs land:
                # back-to-back dummy matmuls on the identity tile
                wm_ps = acc_pool.tile([128, 128], F32, tag="acc",
                                      name="warm")
                for _ in range(84):
                    nc.tensor.matmul(wm_ps[:], ident[:], ident[:],
                                     start=True, stop=True)
            # remaining cos/sin quarters arrive during chunk-0 attention,
            # well before chunk-1 projections need them
            for qq in range(1, 4):
                nc.sync.dma_start(cos_sb[:, qq * QS:(qq + 1) * QS],
                                  ropeCos[:, qq * QS:(qq + 1) * QS])
                nc.sync.dma_start(sin_sb[:, qq * QS:(qq + 1) * QS],
                                  ropeSin[:, qq * QS:(qq + 1) * QS])

            K_ch = []       # per-chunk K, feature-major, duplicated rows
            Q_ch = []       # per-chunk Q, feature-major, [128, 2, SC]
            A_ch = []       # per-chunk normalized attn tiles (2 pairs)
            V1_kb = []      # per key-block token-major [V | 1]

            filler = deque()

            def emit_filler(n=1):
                for _ in range(n):
                    if filler:
                        filler.popleft()()

            def drain_filler():
                while filler:
                    filler.popleft()()

            def proj_q_items(c0, xt):
                """Queue Q projection + RoPE for chunk c0 as filler items."""
                cols = slice(c0 * SC, (c0 + 1) * SC)
                q_sb = qch_pool.tile([128, 2, SC], BF16, tag="q",
                                     name=f"q{c0}")
                Q_ch.append(q_sb)
                for m in range(2):
                    q_ps = acc_pool.tile([128, SC], F32, tag="acc",
                                         name=f"qps{c0}m{m}")
                    box = {}

                    def mm_half(goff, m=m, q_ps=q_ps):
                        for g in range(4):
                            nc.tensor.matmul(
                                q_ps[:],
                                wq_sb[:, goff + g, m * 128:(m + 1) * 128],
                                xt[:, goff + g, :],
                                start=(goff + g == 0), stop=(goff + g == 7))

                    def rope_a(m=m, q_ps=q_ps, box=box):
                        qsw = tmp_pool.tile([128, SC], F32, tag="qsw")
                        t1 = tmp_pool.tile([128, SC], F32, tag="t1")
                        nc.vector.stream_shuffle(qsw[:], q_ps[:], XOR1)
                        nc.vector.tensor_tensor(t1[:], q_ps[:],
                                                cos_sb[:, cols], MUL)
                        box["qsw"], box["t1"] = qsw, t1

                    def rope_b(m=m, box=box):
                        qsw, t1 = box["qsw"], box["t1"]
                        t2 = tmp_pool.tile([128, SC], F32, tag="t2")
                        nc.vector.tensor_tensor(t2[:], qsw[:],
                                                sin_sb[:, cols], MUL)
                        nc.vector.tensor_tensor(q_sb[:, m, :], t1[:], t2[:],
                                                ADD)

                    filler.append(lambda mm_half=mm_half: mm_half(0))
                    filler.append(lambda mm_half=mm_half: mm_half(4))
                    filler.append(rope_a)
                    filler.append(rope_b)

            def proj_kv_items(c0, xt):
                """Queue K/V projection for chunk c0 as filler items."""
                cols = slice(c0 * SC, (c0 + 1) * SC)
                k_sb = kch_pool.tile([128, SC], BF16, tag="k", name=f"k{c0}")
                K_ch.append(k_sb)
                kv_ps = acc_pool.tile([128, SC], F32, tag="acc",
                                      name=f"kvps{c0}")
                v1s = []
                for tb in range(4):
                    v1 = v1_pool.tile([128, 66], BF16, tag="v1",
                                      name=f"v1_{c0}_{tb}")
                    v1s.append(v1)
                    V1_kb.append(v1)
                box = {}

                def mm_half(goff):
                    for g in range(4):
                        nc.tensor.matmul(
                            kv_ps[:], wkv_sb[:, goff + g, :],
                            xt[:, goff + g, :],
                            start=(goff + g == 0), stop=(goff + g == 7))

                def krope_a():
                    ksw = tmp_pool.tile([64, SC], F32, tag="ksw")
                    t1k = tmp_pool.tile([64, SC], F32, tag="t1k")
                    nc.vector.stream_shuffle(ksw[:], kv_ps[0:64, :], XOR1)
                    nc.vector.tensor_tensor(t1k[:], kv_ps[0:64, :],
                                            cos_sb[0:64, cols], MUL)
                    box["ksw"], box["t1k"] = ksw, t1k

                def krope_b():
                    ksw, t1k = box["ksw"], box["t1k"]
                    t2k = tmp_pool.tile([64, SC], F32, tag="t2k")
                    nc.vector.tensor_tensor(t2k[:], ksw[:],
                                            sin_sb[0:64, cols], MUL)
                    nc.vector.tensor_tensor(k_sb[0:64, :], t1k[:], t2k[:], ADD)
                    nc.sync.dma_start(k_sb[64:128, :], k_sb[0:64, :])

                def vcast():
                    vtmp = tmp_pool.tile([128, SC], BF16, tag="vtmp")
                    nc.vector.tensor_copy(vtmp[64:128, :], kv_ps[64:128, :])
                    box["vtmp"] = vtmp

                def vtrans(tb2):
                    vtmp = box["vtmp"]
                    for tb in (tb2, tb2 + 1):
                        v1 = v1s[tb]
                        vt_ps = acc_pool.tile([128, 64], BF16, tag="acc")
                        nc.tensor.transpose(
                            vt_ps[:], vtmp[64:128, tb * 128:(tb + 1) * 128],
                            ident[64:128, 64:128])
                        nc.vector.tensor_copy(v1[:, 0:64], vt_ps[:])
                        nc.vector.memset(v1[:, 64:65], 1.0)

                filler.append(lambda: mm_half(0))
                filler.append(lambda: mm_half(4))
                filler.append(krope_a)
                filler.append(krope_b)
                filler.append(vcast)
                filler.append(lambda: vtrans(0))
                filler.append(lambda: vtrans(2))

            def out_proj_items(c0, hold_tail=False, act_casts=False):
                """Queue the out-projection of chunk c0 as filler items."""
                osb = o_pool.tile([128, 8, SC], BF16, tag="osb",
                                  name=f"osb{c0}")

                def mb_item(mb, on_act=False):
                    o_ps = acc_pool.tile([128, SC], F32, tag="acc")
                    for c in range(2):
                        nc.tensor.matmul(
                            o_ps[:], wo_sb[:, c, mb * 128:(mb + 1) * 128],
                            A_ch[c0][c][:, :], start=(c == 0), stop=(c == 1))
                    if on_act:
                        nc.scalar.copy(osb[:, mb, :], o_ps[:])
                    else:
                        nc.vector.tensor_copy(osb[:, mb, :], o_ps[:])

                # split the store so the first half overlaps the second's MMs
                alt = hold_tail or act_casts
                items = [lambda mb=mb, a=alt: mb_item(mb, a and mb % 2 == 1)
                         for mb in range(8)]
                items.insert(4, lambda: nc.scalar.dma_start(
                    out[:, c0, 0:4, :], osb[:, 0:4, :]))
                items.append(lambda: nc.sync.dma_start(
                    out[:, c0, 4:8, :], osb[:, 4:8, :]))
                if hold_tail:
                    filler.extend(items[:5])
                    return items[5:]
                filler.extend(items)
                return []

            def attention_pair(c0, p):
                nkb = 4 * (c0 + 1)
                nfill = 3 if c0 == 0 else (2 if c0 == 1 else 1)
                op2 = oacc_pool.tile([65, 2, SC], F32, tag="op2")
                pend = None     # (kb, q0, p2) waiting for its AV matmuls

                def av(kb, q0, p2):
                    nc.tensor.matmul(
                        op2[:, 0, q0:], V1_kb[kb][:, 0:65], p2[:, 0, q0:],
                        start=(kb == 0), stop=(kb == nkb - 1))
                    nc.tensor.matmul(
                        op2[:, 1, q0:], V1_kb[kb][:, 0:65], p2[:, 1, q0:],
                        start=(kb == 0), stop=(kb == nkb - 1))

                for kb in range(nkb):
                    kc = K_ch[kb // 4]
                    kcols = slice((kb % 4) * 128, (kb % 4 + 1) * 128)
                    s2 = s_pool.tile([128, 2, SC], F32, tag="s2")
                    nc.tensor.matmul(
                        s2[:, 0, :], kc[0:64, kcols], Q_ch[c0][0:64, p, :],
                        start=True, stop=True)
                    nc.tensor.matmul(
                        s2[:, 1, :], kc[64:128, kcols], Q_ch[c0][64:128, p, :],
                        start=True, stop=True, tile_position=(64, 0))
                    j = kb - (nkb - 4)
                    q0 = 128 * j if j >= 0 else 0
                    if j >= 0:
                        # additive -300 on the strict upper triangle, pre-exp;
                        # two 64-row halves co-run in the two PE row groups
                        nc.tensor.matmul(
                            s2[0:64, :, q0:q0 + 128], ident[0:64, 0:64],
                            mask_sb[0:64, :, :],
                            start=False, stop=True, skip_group_check=True)
                        nc.tensor.matmul(
                            s2[64:128, :, q0:q0 + 128], ident[64:128, 64:128],
                            mask_sb[64:128, :, :],
                            start=False, stop=True, skip_group_check=True,
                            tile_position=(64, 64))
                    p2 = p_pool.tile([128, 2, SC], BF16, tag="p2")
                    nc.scalar.activation(p2[:, :, q0:], s2[:, :, q0:],
                                         EXP, scale=SCALE)
                    # AV lags one kb so the PE never head-of-line blocks on
                    # the exp of the current kb
                    if pend is not None:
                        av(*pend)
                    pend = (kb, q0, p2)
                    emit_filler(nfill)
                av(*pend)
                return op2

            def divide_fast(a_tile, op2):
                """Latency-optimized divide for the final pair: PE rank-1
                broadcast of the bf16 denominator row, approx reciprocal,
                no small-DMA roundtrips in the chain."""
                oc = oc_pool.tile([65, 2, SC], BF16, tag="ocf")
                nc.vector.tensor_copy(oc[:], op2[:])
                pbcA = acc_pool.tile([64, SC], F32, tag="acc")
                pbcB = acc_pool.tile([64, SC], F32, tag="acc")
                nc.tensor.matmul(pbcA[:], ones_b[64:65, :], oc[64:65, 0, :],
                                 start=True, stop=True)
                nc.tensor.matmul(pbcB[:], ones_b[64:65, :], oc[64:65, 1, :],
                                 start=True, stop=True)
                rbcA = bc_pool.tile([64, SC], F32, tag="bc")
                rbcB = bc_pool.tile([64, SC], F32, tag="bc")
                nc.vector.reciprocal_approx_fast(rbcA[:], pbcA[:])
                nc.vector.reciprocal_approx_fast(rbcB[:], pbcB[:])
                nc.vector.tensor_tensor(a_tile[0:64, :], oc[0:64, 0, :],
                                        rbcA[:], MUL)
                tb = bc_pool.tile([64, SC], BF16, tag="tb")
                nc.vector.tensor_tensor(tb[:], oc[0:64, 1, :], rbcB[:], MUL)
                nc.scalar.dma_start(a_tile[64:128, :], tb[:])

            def divide_pair(a_tile, op2, fast=False):
                if fast:
                    return divide_fast(a_tile, op2)
                # evacuate numerator + denominators (frees PSUM banks)
                oc = oc_pool.tile([65, 2, SC], F32, tag="oc")
                nc.vector.tensor_copy(oc[:], op2[:])
                if True:
                    # reshape each [1, 512] sums row to [32, 16] so the
                    # reciprocal runs on many DVE lanes instead of one
                    rsum = r_pool.tile([64, 16], F32, tag="rsum")
                    for hh in range(2):
                        nc.sync.dma_start(
                            rsum[32 * hh: 32 * hh + 32, :],
                            oc[64:65, hh, :].rearrange("o (a n) -> o a n", a=32))
                    rrecs = r_pool.tile([64, 16], F32, tag="rrecs")
                    nc.vector.reciprocal(rrecs[:], rsum[:])
                    rrec = r_pool.tile([1, 2, SC], F32, tag="rrec")
                    for hh in range(2):
                        nc.sync.dma_start(
                            rrec[0:1, hh, :].rearrange("o (a n) -> o a n", a=32),
                            rrecs[32 * hh: 32 * hh + 32, :])
                    rrec_row = rrec[0:1, :, :]
                for hh in range(2):
                    bc = bc_pool.tile([64, SC], F32, tag="bc")
                    nc.gpsimd.partition_broadcast(bc[:], rrec_row[:, hh, :])
                    if hh == 0:
                        nc.vector.tensor_tensor(
                            a_tile[0:64, :], oc[0:64, hh, :], bc[:], MUL)
                    else:
                        tb = bc_pool.tile([64, SC], BF16, tag="tb")
                        nc.vector.tensor_tensor(
                            tb[:], oc[0:64, hh, :], bc[:], MUL)
                        # move to partitions 64:128 (DMA crosses partitions)
                        nc.scalar.dma_start(a_tile[64:128, :], tb[:])

            # chunk 0: interleave q/kv items so K and Q(pair 0) finish first,
            # everything eager (the PE is otherwise idle this early)
            proj_q_items(0, X_ch[0])
            qi = [filler.popleft() for _ in range(len(filler))]
            proj_kv_items(0, X_ch[0])
            kvi = [filler.popleft() for _ in range(len(filler))]
            eager = ([qi[0], kvi[0], qi[1], kvi[1],  # q-m0 + kv MMs
                      kvi[2], kvi[3],                # K RoPE + dup (first:
                      qi[2], qi[3]] +                #  dup hides under q-RoPE)
                     qi[4:8] +                       # q-m1 MMs + RoPE
                     kvi[4:7])                       # V cast + transposes
            for it in eager:
                it()
            nc.scalar.dma_start(wo_sb[:], woP[:])
            xn = None
            pending_outproj = None
            for c0 in range(NCH):
                a_pair = [ach_pool.tile([128, SC], BF16, tag="a",
                                        name=f"a_c{c0}p{i}")
                          for i in range(2)]
                A_ch.append(a_pair)
                if c0 == 0:
                    X_ch.append(load_x(1))
                if c0 + 1 < NCH:
                    xn = X_ch[c0 + 1]
                    proj_q_items(c0 + 1, xn)
                op0 = attention_pair(c0, 0)
                divide_pair(a_pair[0], op0)
                if c0 + 1 < NCH:
                    proj_kv_items(c0 + 1, xn)
                held = []
                if pending_outproj is not None:
                    held = out_proj_items(pending_outproj,
                                          hold_tail=(c0 == NCH - 1))
                op1 = attention_pair(c0, 1)
                for it in held:
                    it()
                divide_pair(a_pair[1], op1, fast=(c0 == NCH - 1))
                if c0 + 2 < NCH:
                    X_ch.append(load_x(c0 + 2))
                # Q/KV of chunk c0+1 must be ready before its attention starts
                drain_filler()
                pending_outproj = c0
            out_proj_items(NCH - 1)
            drain_filler()

    nc.compile()
    return nc


def shard_inputs(x, wq, wk, wv, wo, freqs_cos, freqs_sin):
    """Build the 8 per-core input maps (host-side layout prep)."""
    x = np.ascontiguousarray(np.asarray(x, dtype=np.float32))
    wq = np.asarray(wq, dtype=np.float32)
    wk = np.asarray(wk, dtype=np.float32)
    wv = np.asarray(wv, dtype=np.float32)
    wo = np.asarray(wo, dtype=np.float32)
    cos = np.asarray(freqs_cos, dtype=np.float32)   # [S, 32]
    sin = np.asarray(freqs_sin, dtype=np.float32)

    rope_cos = np.repeat(cos.T, 2, axis=0)          # [64, S]
    rope_sin = np.repeat(sin.T, 2, axis=0)
    rope_sin[0::2, :] *= -1.0                       # row 2i: -sin_i, 2i+1: +sin_i
    rope_cos = np.ascontiguousarray(np.concatenate([rope_cos, rope_cos], 0))
    rope_sin = np.ascontiguousarray(np.concatenate([rope_sin, rope_sin], 0))

    # additive causal mask for a 128x128 diagonal block: -300 where k > q
    kk = np.arange(128)[:, None]
    qq = np.arange(128)[None, :]
    tri = np.where(kk > qq, np.float32(MBIG), np.float32(0.0))
    maskM = np.ascontiguousarray(
        np.broadcast_to(tri[:, None, :], (128, 2, 128))).astype(ml_dtypes.bfloat16)

    in_maps = []
    for core in range(N_CORES):
        b, r = divmod(core, TPG)
        xT = x[b].T                                               # [DM, S]
        # pack so each SBUF partition line is one contiguous DRAM run
        xPm = np.ascontiguousarray(
            xT.reshape(8, 128, NCH, SC).transpose(1, 2, 0, 3))    # [128,NCH,8,SC]
        wq_s = wq[r * KFEAT:(r + 1) * KFEAT]                      # [256, DM]
        wk_s = wk[r * HD:(r + 1) * HD]                            # [64, DM]
        wv_s = wv[r * HD:(r + 1) * HD]
        wkvT = np.concatenate([wk_s, wv_s], axis=0).T             # [DM, 128]
        wqT = wq_s.T                                              # [DM, 256]
        woT = wo[:, r * KFEAT:(r + 1) * KFEAT].T                  # [256, DM]
        wqPm = np.ascontiguousarray(
            wqT.reshape(8, 128, KFEAT).transpose(1, 0, 2))        # [128, 8, 256]
        wkvPm = np.ascontiguousarray(
            wkvT.reshape(8, 128, 128).transpose(1, 0, 2))         # [128, 8, 128]
        woPm = np.ascontiguousarray(
            woT.reshape(2, 128, DM).transpose(1, 0, 2))           # [128, 2, 1024]
        bf = ml_dtypes.bfloat16
        in_maps.append({
            "xP": xPm.astype(bf),
            "wqP": wqPm.astype(bf),
            "wkvP": wkvPm.astype(bf),
            "woP": woPm.astype(bf),
            "ropeCos": rope_cos,
            "ropeSin": rope_sin,
            "maskM": maskM,
        })
    return in_maps


def unshard(results):
    """Sum TP partials per batch, unpack, and transpose to [B, S, DM]."""
    out = np.empty((B, S, DM), dtype=np.float32)
    for b in range(B):
        acc = results[b * TPG]["out"].astype(np.float32)
        for r in range(1, TPG):
            acc = acc + results[b * TPG + r]["out"].astype(np.float32)
        # [128, NCH, 8, SC] -> [DM, S]: row (mb*128+p), col (c*SC+n)
        full = acc.transpose(2, 0, 1, 3).reshape(DM, S)
        out[b] = full.T
    return out


def kernel(**inputs):
    global LAST_RESULTS, _NC_CACHE
    if _NC_CACHE is None:
        _NC_CACHE = build_nc()
    in_maps = shard_inputs(**inputs)
    LAST_RESULTS = run_bass_kernel_spmd(_NC_CACHE, in_maps, list(range(N_CORES)))
    return unshard(LAST_RESULTS.results)


# revision 38
# speedup vs baseline: 1.0002x; 1.0002x over previous
"""GQA attention (B=2, S=2048, DM=1024, H=16, KH=4, RoPE, causal) on 8 TRN2 cores.

Sharding: DP=2 over batch x TP=4 over heads. Core c handles batch c//4 and
q-heads [4r, 4r+4), kv-head r, where r = c % 4. Each core computes a partial
out^T = wo_shard @ attn_shard of shape [DM, S] in bf16; the host sums the 4
partials per batch in f32 and transposes (gather/unshard).

Per-core kernel (single NEFF, SPMD):
  - Q/K feature-major via transposed weight layouts prepared on host; V
    PE-transposed to token-major with a ones column appended (rowsum trick).
  - RoPE: adjacent-partition swap via stream_shuffle + elementwise ops.
  - Causal mask applied PRE-exp as a PE matmul accumulate of -300 onto the
    128x128 diagonal triangles (identity stationary x const triangle moving),
    so exp() output is ~0 there and no post-exp masking is needed.
  - Scores matmuls run two heads concurrently in the two 64-row PE groups
    (K duplicated to partitions 64..127); AV lags one key-block so the PE
    never head-of-line blocks on the current exp. The PE clock (HAM) is
    pre-warmed with dummy matmuls while the first DMAs land.
  - Softmax division: denominator row reshaped across partitions via two
    small DMAs, DVE reciprocal, gpsimd partition_broadcast + multiplies.
  - Projection and out-projection work is chopped into small "filler" items
    that are emitted between attention key-blocks so the PE never runs a
    long non-attention burst while the ACT engine starves.
"""

from collections import deque

import numpy as np
import ml_dtypes

import concourse.bass as bass
import concourse.mybir as mybir
import concourse.tile as tile
from concourse import bacc
from concourse.bass_utils import run_bass_kernel_spmd
from concourse.masks import make_identity

F32 = mybir.dt.float32
BF16 = mybir.dt.bfloat16

B, S, DM, H, KH, HD = 2, 2048, 1024, 16, 4, 64
N_CORES = 8
TPG = 4                 # tensor-parallel group size
QH = H // TPG           # q-heads per core
KFEAT = QH * HD         # 256 q-features per core
SC = 512                # token chunk
NCH = S // SC           # 4
KB = 128                # key block
NKB = S // KB           # 16
SCALE = 1.0 / np.sqrt(HD)
MBIG = -300.0           # pre-exp additive mask value
XOR1 = [i ^ 1 for i in range(32)]

LAST_RESULTS = None     # BassKernelResults of the most recent run (for test.py)
_NC_CACHE = None


def build_nc():
    nc = bacc.Bacc("TRN2", target_bir_lowering=False, debug=False,
                   num_devices=1)

    # all inputs pre-packed on host so each partition line is contiguous
    xP = nc.declare_dram_parameter("xP", [128, NCH, 8, SC], BF16, isOutput=False)
    wqP = nc.declare_dram_parameter("wqP", [128, 8, KFEAT], BF16, isOutput=False)
    wkvP = nc.declare_dram_parameter("wkvP", [128, 8, 128], BF16, isOutput=False)
    woP = nc.declare_dram_parameter("woP", [128, 2, DM], BF16, isOutput=False)
    ropeCos = nc.declare_dram_parameter("ropeCos", [128, S], F32, isOutput=False)
    ropeSin = nc.declare_dram_parameter("ropeSin", [128, S], F32, isOutput=False)
    maskM = nc.declare_dram_parameter("maskM", [128, 2, 128], BF16, isOutput=False)
    out = nc.declare_dram_parameter("out", [128, NCH, 8, SC], BF16, isOutput=True)

    EXP = mybir.ActivationFunctionType.Exp
    MUL = bass.mybir.AluOpType.mult
    ADD = bass.mybir.AluOpType.add

    with tile.TileContext(nc) as tc:
        with (
            tc.tile_pool(name="consts", bufs=1) as consts,
            tc.tile_pool(name="kch", bufs=NCH) as kch_pool,
            tc.tile_pool(name="qch", bufs=NCH) as qch_pool,
            tc.tile_pool(name="ach", bufs=2 * NCH) as ach_pool,
            tc.tile_pool(name="v1p", bufs=NKB) as v1_pool,
            tc.tile_pool(name="xch", bufs=NCH) as xch_pool,
            tc.tile_pool(name="tmp", bufs=3) as tmp_pool,
            tc.tile_pool(name="pp", bufs=8) as p_pool,
            tc.tile_pool(name="ocp", bufs=3) as oc_pool,
            tc.tile_pool(name="rp", bufs=2) as r_pool,
            tc.tile_pool(name="bcp", bufs=4) as bc_pool,
            tc.tile_pool(name="op", bufs=2) as o_pool,
            tc.tile_pool(name="acc", bufs=2, space="PSUM") as acc_pool,
            tc.tile_pool(name="oac", bufs=1, space="PSUM") as oacc_pool,
            tc.tile_pool(name="sme", bufs=2, space="PSUM") as s_pool,
        ):
            # ---- constants (ordered for startup latency) ----
            wq_sb = consts.tile([128, 8, KFEAT], BF16, tag="wq")
            wkv_sb = consts.tile([128, 8, 128], BF16, tag="wkv")
            wo_sb = consts.tile([128, 2, DM], BF16, tag="wo")
            cos_sb = consts.tile([128, S], F32, tag="cos")
            sin_sb = consts.tile([128, S], F32, tag="sin")
            mask_sb = consts.tile([128, 2, 128], BF16, tag="mask")
            ident = consts.tile([128, 128], BF16, tag="ident")
            ones_b = consts.tile([128, 64], BF16, tag="onesb")

            def load_x(c0, eng=None, split=False):
                xt = xch_pool.tile([128, 8, SC], BF16, tag="x",
                                   name=f"x_c{c0}")
                e = eng or nc.sync
                if split:
                    # halves on different HWDGE queues -> parallel transfer
                    e.dma_start(xt[:, 0:4, :], xP[:, c0, 0:4, :])
                    nc.sync.dma_start(xt[:, 4:8, :], xP[:, c0, 4:8, :])
                else:
                    e.dma_start(xt[:], xP[:, c0, :, :])
                return xt

            with tc.high_priority():
                nc.sync.dma_start(wq_sb[:], wqP[:])
                nc.sync.dma_start(wkv_sb[:], wkvP[:])
                X_ch = [load_x(0, eng=nc.scalar, split=True)]
                QS = S // 4
                nc.sync.dma_start(mask_sb[:], maskM[:])
                nc.sync.dma_start(cos_sb[:, 0:QS], ropeCos[:, 0:QS])
                nc.sync.dma_start(sin_sb[:, 0:QS], ropeSin[:, 0:QS])
                make_identity(nc, ident[:])
                nc.vector.memset(ones_b[:], 1.0)
                # warm up the PE clock (HAM) while the first DMAs land:
                # back-to-back dummy matmuls on the identity tile
                wm_ps = acc_pool.tile([128, 128], F32, tag="acc",
                                      name="warm")
                for _ in range(84):
                    nc.tensor.matmul(wm_ps[:], ident[:], ident[:],
                                     start=True, stop=True)
            # remaining cos/sin quarters arrive during chunk-0 attention,
            # well before chunk-1 projections need them
            for qq in range(1, 4):
                nc.sync.dma_start(cos_sb[:, qq * QS:(qq + 1) * QS],
                                  ropeCos[:, qq * QS:(qq + 1) * QS])
                nc.sync.dma_start(sin_sb[:, qq * QS:(qq + 1) * QS],
                                  ropeSin[:, qq * QS:(qq + 1) * QS])

            K_ch = []       # per-chunk K, feature-major, duplicated rows
            Q_ch = []       # per-chunk Q, feature-major, [128, 2, SC]
            A_ch = []       # per-chunk normalized attn tiles (2 pairs)
            V1_kb = []      # per key-block token-major [V | 1]

            filler = deque()

            def emit_filler(n=1):
                for _ in range(n):
                    if filler:
                        filler.popleft()()

            def drain_filler():
                while filler:
                    filler.popleft()()

            def proj_q_items(c0, xt):
                """Queue Q projection + RoPE for chunk c0 as filler items."""
                cols = slice(c0 * SC, (c0 + 1) * SC)
                q_sb = qch_pool.tile([128, 2, SC], BF16, tag="q",
                                     name=f"q{c0}")
                Q_ch.append(q_sb)
                for m in range(2):
                    q_ps = acc_pool.tile([128, SC], F32, tag="acc",
                                         name=f"qps{c0}m{m}")
                    box = {}

                    def mm_half(goff, m=m, q_ps=q_ps):
                        for g in range(4):
                            nc.tensor.matmul(
                                q_ps[:],
                                wq_sb[:, goff + g, m * 128:(m + 1) * 128],
                                xt[:, goff + g, :],
                                start=(goff + g == 0), stop=(goff + g == 7))

                    def rope_a(m=m, q_ps=q_ps, box=box):
                        qsw = tmp_pool.tile([128, SC], F32, tag="qsw")
                        t1 = tmp_pool.tile([128, SC], F32, tag="t1")
                        nc.vector.stream_shuffle(qsw[:], q_ps[:], XOR1)
                        nc.vector.tensor_tensor(t1[:], q_ps[:],
                                                cos_sb[:, cols], MUL)
                        box["qsw"], box["t1"] = qsw, t1

                    def rope_b(m=m, box=box):
                        qsw, t1 = box["qsw"], box["t1"]
                        t2 = tmp_pool.tile([128, SC], F32, tag="t2")
                        nc.vector.tensor_tensor(t2[:], qsw[:],
                                                sin_sb[:, cols], MUL)
                        nc.vector.tensor_tensor(q_sb[:, m, :], t1[:], t2[:],
                                                ADD)

                    filler.append(lambda mm_half=mm_half: mm_half(0))
                    filler.append(lambda mm_half=mm_half: mm_half(4))
                    filler.append(rope_a)
                    filler.append(rope_b)

            def proj_kv_items(c0, xt):
                """Queue K/V projection for chunk c0 as filler items."""
                cols = slice(c0 * SC, (c0 + 1) * SC)
                k_sb = kch_pool.tile([128, SC], BF16, tag="k", name=f"k{c0}")
                K_ch.append(k_sb)
                kv_ps = acc_pool.tile([128, SC], F32, tag="acc",
                                      name=f"kvps{c0}")
                v1s = []
                for tb in range(4):
                    v1 = v1_pool.tile([128, 66], BF16, tag="v1",
                                      name=f"v1_{c0}_{tb}")
                    v1s.append(v1)
                    V1_kb.append(v1)
                box = {}

                def mm_half(goff):
                    for g in range(4):
                        nc.tensor.matmul(
                            kv_ps[:], wkv_sb[:, goff + g, :],
                            xt[:, goff + g, :],
                            start=(goff + g == 0), stop=(goff + g == 7))

                def krope_a():
                    ksw = tmp_pool.tile([64, SC], F32, tag="ksw")
                    t1k = tmp_pool.tile([64, SC], F32, tag="t1k")
                    nc.vector.stream_shuffle(ksw[:], kv_ps[0:64, :], XOR1)
                    nc.vector.tensor_tensor(t1k[:], kv_ps[0:64, :],
                                            cos_sb[0:64, cols], MUL)
                    box["ksw"], box["t1k"] = ksw, t1k

                def krope_b():
                    ksw, t1k = box["ksw"], box["t1k"]
                    t2k = tmp_pool.tile([64, SC], F32, tag="t2k")
                    nc.vector.tensor_tensor(t2k[:], ksw[:],
                                            sin_sb[0:64, cols], MUL)
                    nc.vector.tensor_tensor(k_sb[0:64, :], t1k[:], t2k[:], ADD)
                    nc.sync.dma_start(k_sb[64:128, :], k_sb[0:64, :])

                def vcast():
                    vtmp = tmp_pool.tile([128, SC], BF16, tag="vtmp")
                    nc.vector.tensor_copy(vtmp[64:128, :], kv_ps[64:128, :])
                    box["vtmp"] = vtmp

                def vtrans(tb2):
                    vtmp = box["vtmp"]
                    for tb in (tb2, tb2 + 1):
                        v1 = v1s[tb]
                        vt_ps = acc_pool.tile([128, 64], BF16, tag="acc")
                        nc.tensor.transpose(
                            vt_ps[:], vtmp[64:128, tb * 128:(tb + 1) * 128],
                            ident[64:128, 64:128])
                        nc.vector.tensor_copy(v1[:, 0:64], vt_ps[:])
                        nc.vector.memset(v1[:, 64:65], 1.0)

                filler.append(lambda: mm_half(0))
                filler.append(lambda: mm_half(4))
                filler.append(krope_a)
                filler.append(krope_b)
                filler.append(vcast)
                filler.append(lambda: vtrans(0))
                filler.append(lambda: vtrans(2))

            def out_proj_items(c0, hold_tail=False, act_casts=False):
                """Queue the out-projection of chunk c0 as filler items."""
                osb = o_pool.tile([128, 8, SC], BF16, tag="osb",
                                  name=f"osb{c0}")

                def mb_item(mb, on_act=False):
                    o_ps = acc_pool.tile([128, SC], F32, tag="acc")
                    for c in range(2):
                        nc.tensor.matmul(
                            o_ps[:], wo_sb[:, c, mb * 128:(mb + 1) * 128],
                            A_ch[c0][c][:, :], start=(c == 0), stop=(c == 1))
                    if on_act:
                        nc.scalar.copy(osb[:, mb, :], o_ps[:])
                    else:
                        nc.vector.tensor_copy(osb[:, mb, :], o_ps[:])

                # split the store so the first half overlaps the second's MMs
                alt = hold_tail or act_casts
                items = [lambda mb=mb, a=alt: mb_item(mb, a and mb % 2 == 1)
                         for mb in range(8)]
                items.insert(4, lambda: nc.scalar.dma_start(
                    out[:, c0, 0:4, :], osb[:, 0:4, :]))
                items.append(lambda: nc.sync.dma_start(
                    out[:, c0, 4:8, :], osb[:, 4:8, :]))
                if hold_tail:
                    filler.extend(items[:5])
                    return items[5:]
                filler.extend(items)
                return []

            def attention_pair(c0, p):
                nkb = 4 * (c0 + 1)
                nfill = 3 if c0 == 0 else (2 if c0 == 1 else 1)
                op2 = oacc_pool.tile([65, 2, SC], F32, tag="op2")
                pend = None     # (kb, q0, p2) waiting for its AV matmuls

                def av(kb, q0, p2):
                    nc.tensor.matmul(
                        op2[:, 0, q0:], V1_kb[kb][:, 0:65], p2[:, 0, q0:],
                        start=(kb == 0), stop=(kb == nkb - 1))
                    nc.tensor.matmul(
                        op2[:, 1, q0:], V1_kb[kb][:, 0:65], p2[:, 1, q0:],
                        start=(kb == 0), stop=(kb == nkb - 1))

                for kb in range(nkb):
                    kc = K_ch[kb // 4]
                    kcols = slice((kb % 4) * 128, (kb % 4 + 1) * 128)
                    s2 = s_pool.tile([128, 2, SC], F32, tag="s2")
                    nc.tensor.matmul(
                        s2[:, 0, :], kc[0:64, kcols], Q_ch[c0][0:64, p, :],
                        start=True, stop=True)
                    nc.tensor.matmul(
                        s2[:, 1, :], kc[64:128, kcols], Q_ch[c0][64:128, p, :],
                        start=True, stop=True, tile_position=(64, 0))
                    j = kb - (nkb - 4)
                    q0 = 128 * j if j >= 0 else 0
                    if j >= 0:
                        # additive -300 on the strict upper triangle, pre-exp;
                        # two 64-row halves co-run in the two PE row groups
                        nc.tensor.matmul(
                            s2[0:64, :, q0:q0 + 128], ident[0:64, 0:64],
                            mask_sb[0:64, :, :],
                            start=False, stop=True, skip_group_check=True)
                        nc.tensor.matmul(
                            s2[64:128, :, q0:q0 + 128], ident[64:128, 64:128],
                            mask_sb[64:128, :, :],
                            start=False, stop=True, skip_group_check=True,
                            tile_position=(64, 64))
                    p2 = p_pool.tile([128, 2, SC], BF16, tag="p2")
                    nc.scalar.activation(p2[:, :, q0:], s2[:, :, q0:],
                                         EXP, scale=SCALE)
                    # AV lags one kb so the PE never head-of-line blocks on
                    # the exp of the current kb
                    if pend is not None:
                        av(*pend)
                    pend = (kb, q0, p2)
                    emit_filler(nfill)
                av(*pend)
                return op2

            def divide_fast(a_tile, op2):
                """Latency-optimized divide for the final pair: PE rank-1
                broadcast of the bf16 denominator row, approx reciprocal,
                no small-DMA roundtrips in the chain."""
                oc = oc_pool.tile([65, 2, SC], BF16, tag="ocf")
                nc.vector.tensor_copy(oc[:], op2[:])
                pbcA = acc_pool.tile([64, SC], F32, tag="acc")
                pbcB = acc_pool.tile([64, SC], F32, tag="acc")
                nc.tensor.matmul(pbcA[:], ones_b[64:65, :], oc[64:65, 0, :],
                                 start=True, stop=True)
                nc.tensor.matmul(pbcB[:], ones_b[64:65, :], oc[64:65, 1, :],
                                 start=True, stop=True)
                rbcA = bc_pool.tile([64, SC], F32, tag="bc")
                rbcB = bc_pool.tile([64, SC], F32, tag="bc")
                nc.vector.reciprocal_approx_fast(rbcA[:], pbcA[:])
                nc.vector.reciprocal_approx_fast(rbcB[:], pbcB[:])
                nc.vector.tensor_tensor(a_tile[0:64, :], oc[0:64, 0, :],
                                        rbcA[:], MUL)
                tb = bc_pool.tile([64, SC], BF16, tag="tb")
                nc.vector.tensor_tensor(tb[:], oc[0:64, 1, :], rbcB[:], MUL)
                nc.scalar.dma_start(a_tile[64:128, :], tb[:])

            def divide_pair(a_tile, op2, fast=False):
                if fast:
                    return divide_fast(a_tile, op2)
                # evacuate numerator + denominators (frees PSUM banks)
                oc = oc_pool.tile([65, 2, SC], F32, tag="oc")
                nc.vector.tensor_copy(oc[:], op2[:])
                if True:
                    # reshape each [1, 512] sums row to [32, 16] so the
                    # reciprocal runs on many DVE lanes instead of one
                    rsum = r_pool.tile([64, 16], F32, tag="rsum")
                    for hh in range(2):
                        nc.sync.dma_start(
                            rsum[32 * hh: 32 * hh + 32, :],
                            oc[64:65, hh, :].rearrange("o (a n) -> o a n", a=32))
                    rrecs = r_pool.tile([64, 16], F32, tag="rrecs")
                    nc.vector.reciprocal(rrecs[:], rsum[:])
                    rrec = r_pool.tile([1, 2, SC], F32, tag="rrec")
                    for hh in range(2):
                        nc.sync.dma_start(
                            rrec[0:1, hh, :].rearrange("o (a n) -> o a n", a=32),
                            rrecs[32 * hh: 32 * hh + 32, :])
                    rrec_row = rrec[0:1, :, :]
                for hh in range(2):
                    bc = bc_pool.tile([64, SC], F32, tag="bc")
                    nc.gpsimd.partition_broadcast(bc[:], rrec_row[:, hh, :])
                    if hh == 0:
                        nc.vector.tensor_tensor(
                            a_tile[0:64, :], oc[0:64, hh, :], bc[:], MUL)
                    else:
                        tb = bc_pool.tile([64, SC], BF16, tag="tb")
                        nc.vector.tensor_tensor(
                            tb[:], oc[0:64, hh, :], bc[:], MUL)
                        # move to partitions 64:128 (DMA crosses partitions)
                        nc.scalar.dma_start(a_tile[64:128, :], tb[:])

            # chunk 0: interleave q/kv items so K and Q(pair 0) finish first,
            # everything eager (the PE is otherwise idle this early)
            proj_q_items(0, X_ch[0])
            qi = [filler.popleft() for _ in range(len(filler))]
            proj_kv_items(0, X_ch[0])
            kvi = [filler.popleft() for _ in range(len(filler))]
            eager = ([qi[0], kvi[0], qi[1], kvi[1],  # q-m0 + kv MMs
                      kvi[2], kvi[3],                # K RoPE + dup (first:
                      qi[2], qi[3]] +                #  dup hides under q-RoPE)
                     qi[4:8] +                       # q-m1 MMs + RoPE
                     kvi[4:7])                       # V cast + transposes
            for it in eager:
                it()
            nc.scalar.dma_start(wo_sb[:], woP[:])
            xn = None
            pending_outproj = None
            for c0 in range(NCH):
                a_pair = [ach_pool.tile([128, SC], BF16, tag="a",
                                        name=f"a_c{c0}p{i}")
                          for i in range(2)]
                A_ch.append(a_pair)
                if c0 == 0:
                    X_ch.append(load_x(1))
                if c0 + 1 < NCH:
                    xn = X_ch[c0 + 1]
                    proj_q_items(c0 + 1, xn)
                op0 = attention_pair(c0, 0)
                divide_pair(a_pair[0], op0)
                if c0 + 1 < NCH:
                    proj_kv_items(c0 + 1, xn)
                held = []
                if pending_outproj is not None:
                    held = out_proj_items(pending_outproj,
                                          hold_tail=(c0 == NCH - 1))
                op1 = attention_pair(c0, 1)
                for it in held:
                    it()
                divide_pair(a_pair[1], op1, fast=(c0 == NCH - 1))
                if c0 + 2 < NCH:
                    X_ch.append(load_x(c0 + 2))
                # Q/KV of chunk c0+1 must be ready before its attention starts
                drain_filler()
                pending_outproj = c0
            out_proj_items(NCH - 1)
            drain_filler()

    nc.compile()
    return nc


def shard_inputs(x, wq, wk, wv, wo, freqs_cos, freqs_sin):
    """Build the 8 per-core input maps (host-side layout prep)."""
    x = np.ascontiguousarray(np.asarray(x, dtype=np.float32))
    wq = np.asarray(wq, dtype=np.float32)
    wk = np.asarray(wk, dtype=np.float32)
    wv = np.asarray(wv, dtype=np.float32)
    wo = np.asarray(wo, dtype=np.float32)
    cos = np.asarray(freqs_cos, dtype=np.float32)   # [S, 32]
    sin = np.asarray(freqs_sin, dtype=np.float32)

    rope_cos = np.repeat(cos.T, 2, axis=0)          # [64, S]
    rope_sin = np.repeat(sin.T, 2, axis=0)
    rope_sin[0::2, :] *= -1.0                       # row 2i: -sin_i, 2i+1: +sin_i
    rope_cos = np.ascontiguousarray(np.concatenate([rope_cos, rope_cos], 0))
    rope_sin = np.ascontiguousarray(np.concatenate([rope_sin, rope_sin], 0))

    # additive causal mask for a 128x128 diagonal block: -300 where k > q
    kk = np.arange(128)[:, None]
    qq = np.arange(128)[None, :]
    tri = np.where(kk > qq, np.float32(MBIG), np.float32(0.0))
    maskM = np.ascontiguousarray(
        np.broadcast_to(tri[:, None, :], (128, 2, 128))).astype(ml_dtypes.bfloat16)

    in_maps = []
    for core in range(N_CORES):
        b, r = divmod(core, TPG)
        xT = x[b].T                                               # [DM, S]
        # pack so each SBUF partition line is one contiguous DRAM run
        xPm = np.ascontiguousarray(
            xT.reshape(8, 128, NCH, SC).transpose(1, 2, 0, 3))    # [128,NCH,8,SC]
        wq_s = wq[r * KFEAT:(r + 1) * KFEAT]                      # [256, DM]
        wk_s = wk[r * HD:(r + 1) * HD]                            # [64, DM]
        wv_s = wv[r * HD:(r + 1) * HD]
        wkvT = np.concatenate([wk_s, wv_s], axis=0).T             # [DM, 128]
        wqT = wq_s.T                                              # [DM, 256]
        woT = wo[:, r * KFEAT:(r + 1) * KFEAT].T                  # [256, DM]
        wqPm = np.ascontiguousarray(
            wqT.reshape(8, 128, KFEAT).transpose(1, 0, 2))        # [128, 8, 256]
        wkvPm = np.ascontiguousarray(
            wkvT.reshape(8, 128, 128).transpose(1, 0, 2))         # [128, 8, 128]
        woPm = np.ascontiguousarray(
            woT.reshape(2, 128, DM).transpose(1, 0, 2))           # [128, 2, 1024]
        bf = ml_dtypes.bfloat16
        in_maps.append({
            "xP": xPm.astype(bf),
            "wqP": wqPm.astype(bf),
            "wkvP": wkvPm.astype(bf),
            "woP": woPm.astype(bf),
            "ropeCos": rope_cos,
            "ropeSin": rope_sin,
            "maskM": maskM,
        })
    return in_maps


def unshard(results):
    """Sum TP partials per batch, unpack, and transpose to [B, S, DM]."""
    out = np.empty((B, S, DM), dtype=np.float32)
    for b in range(B):
        acc = results[b * TPG]["out"].astype(np.float32)
        for r in range(1, TPG):
            acc = acc + results[b * TPG + r]["out"].astype(np.float32)
        # [128, NCH, 8, SC] -> [DM, S]: row (mb*128+p), col (c*SC+n)
        full = acc.transpose(2, 0, 1, 3).reshape(DM, S)
        out[b] = full.T
    return out


def kernel(**inputs):
    global LAST_RESULTS, _NC_CACHE
    if _NC_CACHE is None:
        _NC_CACHE = build_nc()
    in_maps = shard_inputs(**inputs)
    LAST_RESULTS = run_bass_kernel_spmd(_NC_CACHE, in_maps, list(range(N_CORES)))
    return unshard(LAST_RESULTS.results)


# revision 40
# speedup vs baseline: 1.0115x; 1.0113x over previous
"""GQA attention (B=2, S=2048, DM=1024, H=16, KH=4, RoPE, causal) on 8 TRN2 cores.

Sharding: DP=2 over batch x TP=4 over heads. Core c handles batch c//4 and
q-heads [4r, 4r+4), kv-head r, where r = c % 4. Each core computes a partial
out^T = wo_shard @ attn_shard of shape [DM, S] in bf16; the host sums the 4
partials per batch in f32 and transposes (gather/unshard).

Per-core kernel (single NEFF, SPMD):
  - Q/K feature-major via transposed weight layouts prepared on host; V
    PE-transposed to token-major with a ones column appended (rowsum trick).
  - RoPE: adjacent-partition swap via stream_shuffle + elementwise ops.
  - Causal mask applied PRE-exp as a PE matmul accumulate of -300 onto the
    128x128 diagonal triangles (identity stationary x const triangle moving),
    so exp() output is ~0 there and no post-exp masking is needed.
  - Scores matmuls run two heads concurrently in the two 64-row PE groups
    (K duplicated to partitions 64..127); AV lags one key-block so the PE
    never head-of-line blocks on the current exp. The PE clock (HAM) is
    pre-warmed with dummy matmuls while the first DMAs land.
  - Softmax division: denominator row reshaped across partitions via two
    small DMAs, DVE reciprocal, gpsimd partition_broadcast + multiplies.
  - Projection and out-projection work is chopped into small "filler" items
    that are emitted between attention key-blocks so the PE never runs a
    long non-attention burst while the ACT engine starves.
"""

from collections import deque

import numpy as np
import ml_dtypes

import concourse.bass as bass
import concourse.mybir as mybir
import concourse.tile as tile
from concourse import bacc
from concourse.bass_utils import run_bass_kernel_spmd
from concourse.masks import make_identity

F32 = mybir.dt.float32
BF16 = mybir.dt.bfloat16

B, S, DM, H, KH, HD = 2, 2048, 1024, 16, 4, 64
N_CORES = 8
TPG = 4                 # tensor-parallel group size
QH = H // TPG           # q-heads per core
KFEAT = QH * HD         # 256 q-features per core
SC = 512                # token chunk
NCH = S // SC           # 4
KB = 128                # key block
NKB = S // KB           # 16
SCALE = 1.0 / np.sqrt(HD)
MBIG = -300.0           # pre-exp additive mask value
XOR1 = [i ^ 1 for i in range(32)]

LAST_RESULTS = None     # BassKernelResults of the most recent run (for test.py)
_NC_CACHE = None


def build_nc():
    nc = bacc.Bacc("TRN2", target_bir_lowering=False, debug=False,
                   num_devices=1)

    # all inputs pre-packed on host so each partition line is contiguous
    xP = nc.declare_dram_parameter("xP", [128, NCH, 8, SC], BF16, isOutput=False)
    wqP = nc.declare_dram_parameter("wqP", [128, 8, KFEAT], BF16, isOutput=False)
    wkvP = nc.declare_dram_parameter("wkvP", [128, 8, 128], BF16, isOutput=False)
    woP = nc.declare_dram_parameter("woP", [128, 2, DM], BF16, isOutput=False)
    ropeCos = nc.declare_dram_parameter("ropeCos", [128, S], F32, isOutput=False)
    ropeSin = nc.declare_dram_parameter("ropeSin", [128, S], F32, isOutput=False)
    maskM = nc.declare_dram_parameter("maskM", [128, 2, 128], BF16, isOutput=False)
    out = nc.declare_dram_parameter("out", [128, NCH, 8, SC], BF16, isOutput=True)

    EXP = mybir.ActivationFunctionType.Exp
    MUL = bass.mybir.AluOpType.mult
    ADD = bass.mybir.AluOpType.add

    with tile.TileContext(nc) as tc:
        with (
            tc.tile_pool(name="consts", bufs=1) as consts,
            tc.tile_pool(name="kch", bufs=NCH) as kch_pool,
            tc.tile_pool(name="qch", bufs=NCH) as qch_pool,
            tc.tile_pool(name="ach", bufs=2 * NCH) as ach_pool,
            tc.tile_pool(name="v1p", bufs=NKB) as v1_pool,
            tc.tile_pool(name="xch", bufs=NCH) as xch_pool,
            tc.tile_pool(name="tmp", bufs=3) as tmp_pool,
            tc.tile_pool(name="pp", bufs=8) as p_pool,
            tc.tile_pool(name="ocp", bufs=3) as oc_pool,
            tc.tile_pool(name="rp", bufs=2) as r_pool,
            tc.tile_pool(name="bcp", bufs=4) as bc_pool,
            tc.tile_pool(name="op", bufs=2) as o_pool,
            tc.tile_pool(name="acc", bufs=2, space="PSUM") as acc_pool,
            tc.tile_pool(name="oac", bufs=1, space="PSUM") as oacc_pool,
            tc.tile_pool(name="sme", bufs=2, space="PSUM") as s_pool,
        ):
            # ---- constants (ordered for startup latency) ----
            wq_sb = consts.tile([128, 8, KFEAT], BF16, tag="wq")
            wkv_sb = consts.tile([128, 8, 128], BF16, tag="wkv")
            wo_sb = consts.tile([128, 2, DM], BF16, tag="wo")
            cos_sb = consts.tile([128, S], F32, tag="cos")
            sin_sb = consts.tile([128, S], F32, tag="sin")
            mask_sb = consts.tile([128, 2, 128], BF16, tag="mask")
            ident = consts.tile([128, 128], BF16, tag="ident")
            ones_b = consts.tile([128, 64], BF16, tag="onesb")

            def load_x(c0, eng=None, split=False):
                xt = xch_pool.tile([128, 8, SC], BF16, tag="x",
                                   name=f"x_c{c0}")
                e = eng or nc.sync
                if split:
                    # halves on different HWDGE queues -> parallel transfer
                    e.dma_start(xt[:, 0:4, :], xP[:, c0, 0:4, :])
                    nc.sync.dma_start(xt[:, 4:8, :], xP[:, c0, 4:8, :])
                else:
                    e.dma_start(xt[:], xP[:, c0, :, :])
                return xt

            with tc.high_priority():
                nc.sync.dma_start(wq_sb[:], wqP[:])
                nc.sync.dma_start(wkv_sb[:], wkvP[:])
                X_ch = [load_x(0, eng=nc.scalar, split=True)]
                QS = S // 4
                nc.sync.dma_start(mask_sb[:], maskM[:])
                nc.sync.dma_start(cos_sb[:, 0:QS], ropeCos[:, 0:QS])
                nc.sync.dma_start(sin_sb[:, 0:QS], ropeSin[:, 0:QS])
                make_identity(nc, ident[:])
                nc.vector.memset(ones_b[:], 1.0)
                # warm up the PE clock (HAM) while the first DMAs land:
                # back-to-back dummy matmuls on the identity tile
                wm_ps = acc_pool.tile([128, 128], F32, tag="acc",
                                      name="warm")
                for _ in range(84):
                    nc.tensor.matmul(wm_ps[:], ident[:], ident[:],
                                     start=True, stop=True)
            # remaining cos/sin quarters arrive during chunk-0 attention,
            # well before chunk-1 projections need them
            for qq in range(1, 4):
                nc.sync.dma_start(cos_sb[:, qq * QS:(qq + 1) * QS],
                                  ropeCos[:, qq * QS:(qq + 1) * QS])
                nc.sync.dma_start(sin_sb[:, qq * QS:(qq + 1) * QS],
                                  ropeSin[:, qq * QS:(qq + 1) * QS])

            K_ch = []       # per-chunk K, feature-major, duplicated rows
            Q_ch = []       # per-chunk Q, feature-major, [128, 2, SC]
            A_ch = []       # per-chunk normalized attn tiles (2 pairs)
            V1_kb = []      # per key-block token-major [V | 1]

            filler = deque()

            def emit_filler(n=1):
                for _ in range(n):
                    if filler:
                        filler.popleft()()

            def drain_filler():
                while filler:
                    filler.popleft()()

            def proj_q_items(c0, xt):
                """Queue Q projection + RoPE for chunk c0 as filler items."""
                cols = slice(c0 * SC, (c0 + 1) * SC)
                q_sb = qch_pool.tile([128, 2, SC], BF16, tag="q",
                                     name=f"q{c0}")
                Q_ch.append(q_sb)
                for m in range(2):
                    q_ps = acc_pool.tile([128, SC], F32, tag="acc",
                                         name=f"qps{c0}m{m}")
                    box = {}

                    def mm_half(goff, m=m, q_ps=q_ps):
                        for g in range(4):
                            nc.tensor.matmul(
                                q_ps[:],
                                wq_sb[:, goff + g, m * 128:(m + 1) * 128],
                                xt[:, goff + g, :],
                                start=(goff + g == 0), stop=(goff + g == 7))

                    def rope_a(m=m, q_ps=q_ps, box=box):
                        qsw = tmp_pool.tile([128, SC], F32, tag="qsw")
                        t1 = tmp_pool.tile([128, SC], F32, tag="t1")
                        nc.vector.stream_shuffle(qsw[:], q_ps[:], XOR1)
                        nc.vector.tensor_tensor(t1[:], q_ps[:],
                                                cos_sb[:, cols], MUL)
                        box["qsw"], box["t1"] = qsw, t1

                    def rope_b(m=m, box=box):
                        qsw, t1 = box["qsw"], box["t1"]
                        t2 = tmp_pool.tile([128, SC], F32, tag="t2")
                        nc.vector.tensor_tensor(t2[:], qsw[:],
                                                sin_sb[:, cols], MUL)
                        nc.vector.tensor_tensor(q_sb[:, m, :], t1[:], t2[:],
                                                ADD)

                    filler.append(lambda mm_half=mm_half: mm_half(0))
                    filler.append(lambda mm_half=mm_half: mm_half(4))
                    filler.append(rope_a)
                    filler.append(rope_b)

            def proj_kv_items(c0, xt):
                """Queue K/V projection for chunk c0 as filler items."""
                cols = slice(c0 * SC, (c0 + 1) * SC)
                k_sb = kch_pool.tile([128, SC], BF16, tag="k", name=f"k{c0}")
                K_ch.append(k_sb)
                kv_ps = acc_pool.tile([128, SC], F32, tag="acc",
                                      name=f"kvps{c0}")
                v1s = []
                for tb in range(4):
                    v1 = v1_pool.tile([128, 66], BF16, tag="v1",
                                      name=f"v1_{c0}_{tb}")
                    v1s.append(v1)
                    V1_kb.append(v1)
                box = {}

                def mm_half(goff):
                    for g in range(4):
                        nc.tensor.matmul(
                            kv_ps[:], wkv_sb[:, goff + g, :],
                            xt[:, goff + g, :],
                            start=(goff + g == 0), stop=(goff + g == 7))

                def krope_a():
                    ksw = tmp_pool.tile([64, SC], F32, tag="ksw")
                    t1k = tmp_pool.tile([64, SC], F32, tag="t1k")
                    nc.vector.stream_shuffle(ksw[:], kv_ps[0:64, :], XOR1)
                    nc.vector.tensor_tensor(t1k[:], kv_ps[0:64, :],
                                            cos_sb[0:64, cols], MUL)
                    box["ksw"], box["t1k"] = ksw, t1k

                def krope_b():
                    ksw, t1k = box["ksw"], box["t1k"]
                    t2k = tmp_pool.tile([64, SC], F32, tag="t2k")
                    nc.vector.tensor_tensor(t2k[:], ksw[:],
                                            sin_sb[0:64, cols], MUL)
                    nc.vector.tensor_tensor(k_sb[0:64, :], t1k[:], t2k[:], ADD)
                    nc.sync.dma_start(k_sb[64:128, :], k_sb[0:64, :])

                def vcast():
                    vtmp = tmp_pool.tile([128, SC], BF16, tag="vtmp")
                    nc.vector.tensor_copy(vtmp[64:128, :], kv_ps[64:128, :])
                    box["vtmp"] = vtmp

                def vtrans(tb2):
                    vtmp = box["vtmp"]
                    for tb in (tb2, tb2 + 1):
                        v1 = v1s[tb]
                        vt_ps = acc_pool.tile([128, 64], BF16, tag="acc")
                        nc.tensor.transpose(
                            vt_ps[:], vtmp[64:128, tb * 128:(tb + 1) * 128],
                            ident[64:128, 64:128])
                        nc.vector.tensor_copy(v1[:, 0:64], vt_ps[:])
                        nc.vector.memset(v1[:, 64:65], 1.0)

                filler.append(lambda: mm_half(0))
                filler.append(lambda: mm_half(4))
                filler.append(krope_a)
                filler.append(krope_b)
                filler.append(vcast)
                filler.append(lambda: vtrans(0))
                filler.append(lambda: vtrans(2))

            def out_proj_items(c0, hold_tail=False, act_casts=False):
                """Queue the out-projection of chunk c0 as filler items."""
                osb = o_pool.tile([128, 8, SC], BF16, tag="osb",
                                  name=f"osb{c0}")

                def mb_item(mb, on_act=False):
                    o_ps = acc_pool.tile([128, SC], F32, tag="acc")
                    for c in range(2):
                        nc.tensor.matmul(
                            o_ps[:], wo_sb[:, c, mb * 128:(mb + 1) * 128],
                            A_ch[c0][c][:, :], start=(c == 0), stop=(c == 1))
                    if on_act:
                        nc.scalar.copy(osb[:, mb, :], o_ps[:])
                    else:
                        nc.vector.tensor_copy(osb[:, mb, :], o_ps[:])

                # split the store so the first half overlaps the second's MMs
                alt = hold_tail or act_casts
                items = [lambda mb=mb, a=alt: mb_item(mb, a and mb % 2 == 1)
                         for mb in range(8)]
                items.insert(4, lambda: nc.scalar.dma_start(
                    out[:, c0, 0:4, :], osb[:, 0:4, :]))
                items.append(lambda: nc.sync.dma_start(
                    out[:, c0, 4:8, :], osb[:, 4:8, :]))
                if hold_tail:
                    filler.extend(items[:5])
                    return items[5:]
                filler.extend(items)
                return []

            def attention_pair(c0, p):
                nkb = 4 * (c0 + 1)
                nfill = 3 if c0 == 0 else (2 if c0 == 1 else 1)
                op2 = oacc_pool.tile([65, 2, SC], F32, tag="op2")
                pend = None     # (kb, q0, p2) waiting for its AV matmuls

                def av(kb, q0, p2):
                    nc.tensor.matmul(
                        op2[:, 0, q0:], V1_kb[kb][:, 0:65], p2[:, 0, q0:],
                        start=(kb == 0), stop=(kb == nkb - 1))
                    nc.tensor.matmul(
                        op2[:, 1, q0:], V1_kb[kb][:, 0:65], p2[:, 1, q0:],
                        start=(kb == 0), stop=(kb == nkb - 1))

                for kb in range(nkb):
                    kc = K_ch[kb // 4]
                    kcols = slice((kb % 4) * 128, (kb % 4 + 1) * 128)
                    s2 = s_pool.tile([128, 2, SC], F32, tag="s2")
                    nc.tensor.matmul(
                        s2[:, 0, :], kc[0:64, kcols], Q_ch[c0][0:64, p, :],
                        start=True, stop=True)
                    nc.tensor.matmul(
                        s2[:, 1, :], kc[64:128, kcols], Q_ch[c0][64:128, p, :],
                        start=True, stop=True, tile_position=(64, 0))
                    j = kb - (nkb - 4)
                    q0 = 128 * j if j >= 0 else 0
                    if j >= 0:
                        # additive -300 on the strict upper triangle, pre-exp;
                        # two 64-row halves co-run in the two PE row groups
                        nc.tensor.matmul(
                            s2[0:64, :, q0:q0 + 128], ident[0:64, 0:64],
                            mask_sb[0:64, :, :],
                            start=False, stop=True, skip_group_check=True)
                        nc.tensor.matmul(
                            s2[64:128, :, q0:q0 + 128], ident[64:128, 64:128],
                            mask_sb[64:128, :, :],
                            start=False, stop=True, skip_group_check=True,
                            tile_position=(64, 64))
                    p2 = p_pool.tile([128, 2, SC], BF16, tag="p2")
                    nc.scalar.activation(p2[:, :, q0:], s2[:, :, q0:],
                                         EXP, scale=SCALE)
                    # AV lags one kb so the PE never head-of-line blocks on
                    # the exp of the current kb
                    if pend is not None:
                        av(*pend)
                    pend = (kb, q0, p2)
                    emit_filler(nfill)
                av(*pend)
                return op2

            def divide_fast(a_tile, op2):
                """Latency-optimized divide for the final pair: PE rank-1
                broadcast of the bf16 denominator row, approx reciprocal,
                no small-DMA roundtrips in the chain."""
                oc = oc_pool.tile([65, 2, SC], BF16, tag="ocf")
                nc.vector.tensor_copy(oc[:], op2[:])
                pbcA = acc_pool.tile([64, SC], F32, tag="acc")
                pbcB = acc_pool.tile([64, SC], F32, tag="acc")
                nc.tensor.matmul(pbcA[:], ones_b[64:65, :], oc[64:65, 0, :],
                                 start=True, stop=True)
                nc.tensor.matmul(pbcB[:], ones_b[64:65, :], oc[64:65, 1, :],
                                 start=True, stop=True)
                rbcA = bc_pool.tile([64, SC], F32, tag="bc")
                rbcB = bc_pool.tile([64, SC], F32, tag="bc")
                nc.vector.reciprocal_approx_fast(rbcA[:], pbcA[:])
                nc.vector.reciprocal_approx_fast(rbcB[:], pbcB[:])
                nc.vector.tensor_tensor(a_tile[0:64, :], oc[0:64, 0, :],
                                        rbcA[:], MUL)
                tb = bc_pool.tile([64, SC], BF16, tag="tb")
                nc.vector.tensor_tensor(tb[:], oc[0:64, 1, :], rbcB[:], MUL)
                nc.scalar.dma_start(a_tile[64:128, :], tb[:])

            def divide_pair(a_tile, op2, fast=False):
                if fast:
                    return divide_fast(a_tile, op2)
                # evacuate numerator + denominators (frees PSUM banks)
                oc = oc_pool.tile([65, 2, SC], F32, tag="oc")
                nc.vector.tensor_copy(oc[:], op2[:])
                if True:
                    # reshape each [1, 512] sums row to [32, 16] so the
                    # reciprocal runs on many DVE lanes instead of one
                    rsum = r_pool.tile([64, 16], F32, tag="rsum")
                    for hh in range(2):
                        nc.sync.dma_start(
                            rsum[32 * hh: 32 * hh + 32, :],
                            oc[64:65, hh, :].rearrange("o (a n) -> o a n", a=32))
                    rrecs = r_pool.tile([64, 16], F32, tag="rrecs")
                    nc.vector.reciprocal(rrecs[:], rsum[:])
                    rrec = r_pool.tile([1, 2, SC], F32, tag="rrec")
                    for hh in range(2):
                        nc.sync.dma_start(
                            rrec[0:1, hh, :].rearrange("o (a n) -> o a n", a=32),
                            rrecs[32 * hh: 32 * hh + 32, :])
                    rrec_row = rrec[0:1, :, :]
                for hh in range(2):
                    bc = bc_pool.tile([64, SC], F32, tag="bc")
                    nc.gpsimd.partition_broadcast(bc[:], rrec_row[:, hh, :])
                    if hh == 0:
                        nc.vector.tensor_tensor(
                            a_tile[0:64, :], oc[0:64, hh, :], bc[:], MUL)
                    else:
                        tb = bc_pool.tile([64, SC], BF16, tag="tb")
                        nc.vector.tensor_tensor(
                            tb[:], oc[0:64, hh, :], bc[:], MUL)
                        # move to partitions 64:128 (DMA crosses partitions)
                        nc.scalar.dma_start(a_tile[64:128, :], tb[:])

            # chunk 0: interleave q/kv items so K and Q(pair 0) finish first,
            # everything eager (the PE is otherwise idle this early)
            proj_q_items(0, X_ch[0])
            qi = [filler.popleft() for _ in range(len(filler))]
            proj_kv_items(0, X_ch[0])
            kvi = [filler.popleft() for _ in range(len(filler))]
            eager = ([qi[0], kvi[0], qi[1], kvi[1],  # q-m0 + kv MMs
                      kvi[2], kvi[3],                # K RoPE + dup (first:
                      qi[2], qi[3]] +                #  dup hides under q-RoPE)
                     qi[4:8] +                       # q-m1 MMs + RoPE
                     kvi[4:7])                       # V cast + transposes
            for it in eager:
                it()
            nc.scalar.dma_start(wo_sb[:], woP[:])
            xn = None
            pending_outproj = None
            for c0 in range(NCH):
                a_pair = [ach_pool.tile([128, SC], BF16, tag="a",
                                        name=f"a_c{c0}p{i}")
                          for i in range(2)]
                A_ch.append(a_pair)
                if c0 == 0:
                    X_ch.append(load_x(1))
                if c0 + 1 < NCH:
                    xn = X_ch[c0 + 1]
                    proj_q_items(c0 + 1, xn)
                op0 = attention_pair(c0, 0)
                divide_pair(a_pair[0], op0)
                if c0 + 1 < NCH:
                    proj_kv_items(c0 + 1, xn)
                held = []
                if pending_outproj is not None:
                    held = out_proj_items(pending_outproj,
                                          hold_tail=(c0 == NCH - 1))
                op1 = attention_pair(c0, 1)
                for it in held:
                    it()
                divide_pair(a_pair[1], op1, fast=(c0 == NCH - 1))
                if c0 + 2 < NCH:
                    X_ch.append(load_x(c0 + 2))
                # Q/KV of chunk c0+1 must be ready before its attention starts
                drain_filler()
                pending_outproj = c0
            out_proj_items(NCH - 1)
            drain_filler()

    nc.compile()
    return nc


def shard_inputs(x, wq, wk, wv, wo, freqs_cos, freqs_sin):
    """Build the 8 per-core input maps (host-side layout prep)."""
    x = np.ascontiguousarray(np.asarray(x, dtype=np.float32))
    wq = np.asarray(wq, dtype=np.float32)
    wk = np.asarray(wk, dtype=np.float32)
    wv = np.asarray(wv, dtype=np.float32)
    wo = np.asarray(wo, dtype=np.float32)
    cos = np.asarray(freqs_cos, dtype=np.float32)   # [S, 32]
    sin = np.asarray(freqs_sin, dtype=np.float32)

    rope_cos = np.repeat(cos.T, 2, axis=0)          # [64, S]
    rope_sin = np.repeat(sin.T, 2, axis=0)
    rope_sin[0::2, :] *= -1.0                       # row 2i: -sin_i, 2i+1: +sin_i
    rope_cos = np.ascontiguousarray(np.concatenate([rope_cos, rope_cos], 0))
    rope_sin = np.ascontiguousarray(np.concatenate([rope_sin, rope_sin], 0))

    # additive causal mask for a 128x128 diagonal block: -300 where k > q
    kk = np.arange(128)[:, None]
    qq = np.arange(128)[None, :]
    tri = np.where(kk > qq, np.float32(MBIG), np.float32(0.0))
    maskM = np.ascontiguousarray(
        np.broadcast_to(tri[:, None, :], (128, 2, 128))).astype(ml_dtypes.bfloat16)

    in_maps = []
    for core in range(N_CORES):
        b, r = divmod(core, TPG)
        xT = x[b].T                                               # [DM, S]
        # pack so each SBUF partition line is one contiguous DRAM run
        xPm = np.ascontiguousarray(
            xT.reshape(8, 128, NCH, SC).transpose(1, 2, 0, 3))    # [128,NCH,8,SC]
        wq_s = wq[r * KFEAT:(r + 1) * KFEAT]                      # [256, DM]
        wk_s = wk[r * HD:(r + 1) * HD]                            # [64, DM]
        wv_s = wv[r * HD:(r + 1) * HD]
        wkvT = np.concatenate([wk_s, wv_s], axis=0).T             # [DM, 128]
        wqT = wq_s.T                                              # [DM, 256]
        woT = wo[:, r * KFEAT:(r + 1) * KFEAT].T                  # [256, DM]
        wqPm = np.ascontiguousarray(
            wqT.reshape(8, 128, KFEAT).transpose(1, 0, 2))        # [128, 8, 256]
        wkvPm = np.ascontiguousarray(
            wkvT.reshape(8, 128, 128).transpose(1, 0, 2))         # [128, 8, 128]
        woPm = np.ascontiguousarray(
            woT.reshape(2, 128, DM).transpose(1, 0, 2))           # [128, 2, 1024]
        bf = ml_dtypes.bfloat16
        in_maps.append({
            "xP": xPm.astype(bf),
            "wqP": wqPm.astype(bf),
            "wkvP": wkvPm.astype(bf),
            "woP": woPm.astype(bf),
            "ropeCos": rope_cos,
            "ropeSin": rope_sin,
            "maskM": maskM,
        })
    return in_maps


def unshard(results):
    """Sum TP partials per batch, unpack, and transpose to [B, S, DM]."""
    out = np.empty((B, S, DM), dtype=np.float32)
    for b in range(B):
        acc = results[b * TPG]["out"].astype(np.float32)
        for r in range(1, TPG):
            acc = acc + results[b * TPG + r]["out"].astype(np.float32)
        # [128, NCH, 8, SC] -> [DM, S]: row (mb*128+p), col (c*SC+n)
        full = acc.transpose(2, 0, 1, 3).reshape(DM, S)
        out[b] = full.T
    return out


def kernel(**inputs):
    global LAST_RESULTS, _NC_CACHE
    if _NC_CACHE is None:
        _NC_CACHE = build_nc()
    in_maps = shard_inputs(**inputs)
    LAST_RESULTS = run_bass_kernel_spmd(_NC_CACHE, in_maps, list(range(N_CORES)))
    return unshard(LAST_RESULTS.results)
